# revision 16
# baseline (speedup 1.0000x reference)
"""Banded causal self-attention (sparse_attention) for 8 trn2 NeuronCores.

Sharding: tensor-parallel over head groups (4 groups x 4 heads of dim 64)
x data-parallel over batch (2). Core c handles batch c//4, head group c%4.
Each core computes a partial output projection; the host sums the 4 group
partials per batch (partials are written as fp16 to halve output DMA).

v12 schedule (all matmul operands fp16, PSUM f32):
  A: qkT[512, T] = W_qk.T @ x.T   -- k-outer, 8 concurrent PSUM chains so
     matmuls stream while the xT chunks are still DMA-ing in (keeps the PE
     HAM clock warm from ~2us onward).
  B: v[T, 256] = x @ W_v (+ ones column for the softmax denominator).
  C: per head pair (row bases 0/64 of shared tiles so score matmuls of a
     pair run on disjoint PE row groups): banded scores for a 256-query
     superblock live in one [128, 1536] PSUM tile (3 banks, region layout
     [b|a|d|c] per head chosen so no matmul output crosses a bank).
     The causal/band masks are applied as additive -30000 matmuls against
     precomputed triangular lhsT tiles with identity rhs (no gpsimd/DVE in
     the chain), then ONE exp activation per (sb, pair) covers both heads.
     att@v accumulates [v|1] so row 64 of yts is the softmax denominator.
  D: reciprocal of the denominators via a [32,128] bounce tile (DVE recip
     is 8 cyc/elem; never run it on a [1, T] row), broadcast via K=1
     matmuls pair-packed into [128, 512] PSUM, normalize on DVE.
  E: out = y_norm @ W_p, K=128 per pair, accumulated over the 2 pairs;
     fp16 partials stream to DRAM per 128-row block.
"""

import numpy as np

B, T, C = 2, 2048, 1024
N_HEAD = 16
MEMORY = 256
D = 64           # head dim
G = 4            # head groups (tensor parallel)
HPG = 4          # heads per group
GC = HPG * D     # 256 columns per group
N_CORES = 8
TB = T // 128    # 16 row blocks
SB = T // 256    # 8 query super-blocks
NEG = -30000.0   # additive mask; exp(0.125 * -30000) == 0 in f32

# aux tensor column layout
AUX_ONES = 0          # [:, 0:68] ones (vplus ones col + bc ones row)
AUX_I = 68            # [:, 68:196] identity, [:, 196:324] identity (I2)
AUX_UP = 324          # [:, 324:452] lhsT for "keep p>=f" mask (strict lower tri = NEG)
AUX_LO = 452          # [:, 452:580] lhsT for "keep p<=f" mask (strict upper tri = NEG)
AUX_W = 580

_PROGRAM_CACHE = {}


def _emit(tc, nc, xT, wqk, wv, wp, aux, out, debug=None):
    import concourse.mybir as mybir

    f32 = mybir.dt.float32
    f16 = mybir.dt.float16

    from contextlib import ExitStack

    ctx = ExitStack()
    with ctx:
        const = ctx.enter_context(tc.tile_pool(name="const", bufs=1))
        wpool = ctx.enter_context(tc.tile_pool(name="wpool", bufs=1))
        xpool = ctx.enter_context(tc.tile_pool(name="xpool", bufs=1))
        qkt_pool = ctx.enter_context(tc.tile_pool(name="qkt", bufs=1))
        vplus_pool = ctx.enter_context(tc.tile_pool(name="vplus", bufs=1))
        expst_pool = ctx.enter_context(tc.tile_pool(name="expst", bufs=3))
        ytpool = ctx.enter_context(tc.tile_pool(name="yt", bufs=1))
        outsb_pool = ctx.enter_context(tc.tile_pool(name="outsb", bufs=3))
        # PSUM: "st" 2 x 3 banks + "bank" 2 x 1 bank = 8 banks
        ps_st = ctx.enter_context(tc.tile_pool(name="ps_st", bufs=2, space="PSUM"))
        ps_bk = ctx.enter_context(tc.tile_pool(name="ps_bk", bufs=2, space="PSUM"))

        # ---- aux constants (ones / identity / mask triangles) ----
        aux_sb = const.tile([128, AUX_W], f16, name="aux", tag="aux")
        nc.gpsimd.dma_start(aux_sb[:], aux[:])
        ones_row = aux_sb[0:1, 0:64]
        ident = aux_sb[:, AUX_I:AUX_I + 128]
        ident2 = aux_sb[:, AUX_I:AUX_I + 256]
        mask_up = aux_sb[:, AUX_UP:AUX_UP + 128]
        mask_lo = aux_sb[:, AUX_LO:AUX_LO + 128]

        # ---- HAM pre-warm: dummy matmuls on zeros while input DMA runs ----
        # The PE clock sits at 1.2 GHz until ~3.4us of sustained activity;
        # burn the DMA prefix warming it so phase A runs at 2.4 GHz.
        wz = const.tile([128, 512], f16, name="warmz", tag="warmz")
        nc.vector.memzero(wz[:])
        warm_ps = ps_bk.tile([128, 512], f32, name="psW", tag="bank")
        for i in range(18):
            nc.tensor.matmul(warm_ps[:], wz[:, 0:128], wz[:],
                             start=True, stop=True, skip_group_check=True)

        # ---- input loads: xT + wqk chunk-interleaved (phase A streams) ----
        xT_sb, wqk_sb, wv_sb = [], [], []
        for k in range(8):
            qa = nc.sync if k % 2 == 0 else nc.scalar
            qb = nc.scalar if k % 2 == 0 else nc.sync
            t = xpool.tile([128, T], f16, name=f"xT{k}", tag=f"xT{k}")
            qa.dma_start(t[:], xT[k * 128:(k + 1) * 128, :])
            xT_sb.append(t)
            t = wpool.tile([128, 2 * GC], f16, name=f"wqk{k}", tag=f"wqk{k}")
            qb.dma_start(t[:], wqk[k * 128:(k + 1) * 128, :])
            wqk_sb.append(t)

        # ---- phase A: qkT[512, T], k-outer with 8 concurrent chains ----
        qkT_sb = []
        for m in range(4):
            t = qkt_pool.tile([128, T], f16, name=f"qkT{m}", tag=f"qkT{m}")
            qkT_sb.append(t)

        def emit_A(ms):
            # 8 chains: (m in ms) x (t4 in 0..3); chain (m, t4) lives in a
            # fixed 512-col region of an st slot (3 chains) or a bank slot.
            st0 = ps_st.tile([128, 1536], f32, name="psA0", tag="st")
            st1 = ps_st.tile([128, 1536], f32, name="psA1", tag="st")
            bk0 = ps_bk.tile([128, 512], f32, name="psA2", tag="bank")
            bk1 = ps_bk.tile([128, 512], f32, name="psA3", tag="bank")
            regions = {}
            for mi, m in enumerate(ms):
                stt, bkt = (st0, bk0) if mi == 0 else (st1, bk1)
                for t4 in range(4):
                    if t4 < 3:
                        regions[(m, t4)] = stt[:, t4 * 512:(t4 + 1) * 512]
                    else:
                        regions[(m, t4)] = bkt[:]
            for k in range(8):
                for m in ms:
                    for t4 in range(4):
                        nc.tensor.matmul(
                            regions[(m, t4)],
                            wqk_sb[k][:, m * 128:(m + 1) * 128],
                            xT_sb[k][:, t4 * 512:(t4 + 1) * 512],
                            start=(k == 0),
                            stop=(k == 7),
                        )
            for i, ((m, t4), reg) in enumerate(regions.items()):
                eng = nc.scalar.copy if i % 2 == 0 else nc.vector.tensor_copy
                eng(qkT_sb[m][:, t4 * 512:(t4 + 1) * 512], reg)

        emit_A((0, 2))

        # weights for later phases land while A part 2 computes
        for k in range(8):
            qb = nc.scalar if k % 2 == 0 else nc.sync
            t = wpool.tile([128, GC], f16, name=f"wv{k}", tag=f"wv{k}")
            qb.dma_start(t[:], wv[k * 128:(k + 1) * 128, :])
            wv_sb.append(t)
        wp_sb = []
        for pr in range(2):
            t = wpool.tile([128, C], f16, name=f"wp{pr}", tag=f"wp{pr}")
            nc.gpsimd.dma_start(t[:], wp[pr * 128:(pr + 1) * 128, :])
            wp_sb.append(t)

        emit_A((1, 3))

        # ---- phase B: v[T, 256] (+ ones col) ----
        vplus_sb = []
        for tb in range(TB):
            ps = ps_bk.tile([128, GC], f32, name="psB", tag="bank")
            for k in range(8):
                nc.tensor.matmul(
                    ps[:],
                    xT_sb[k][:, tb * 128:(tb + 1) * 128],
                    wv_sb[k][:],
                    start=(k == 0),
                    stop=(k == 7),
                )
            vp = vplus_pool.tile([128, HPG, D + 1], f16, name=f"vplus{tb}",
                                 tag=f"vplus{tb}")
            eng = nc.scalar.copy if tb % 2 == 0 else nc.vector.tensor_copy
            eng(vp[:, :, 0:D], ps[:].rearrange("p (h d) -> p h d", h=HPG))
            q = nc.sync if tb % 2 == 0 else nc.scalar
            q.dma_start(
                vp[:, :, D:D + 1],
                aux[:, 0:HPG].rearrange("p (h o) -> p h o", o=1),
            )
            vplus_sb.append(vp)

        # per-head views into qkT: q rows = h*64.., k rows = 256 + h*64..
        def qT_h(h):
            return qkT_sb[h // 2][(h % 2) * 64:(h % 2) * 64 + 64, :]

        def kT_h(h):
            return qkT_sb[2 + h // 2][(h % 2) * 64:(h % 2) * 64 + 64, :]

        # region layout per head within the [128, 1536] score tile:
        #   b: +0:256 (tkb=2sb-1, all 256 queries; right half masked UP)
        #   a: +256:384 (tkb=2sb-2, queries 0:128; masked UP)
        #   d: +384:512 (tkb=2sb+1, queries 128:256; masked LO)
        #   c: +512:768 (tkb=2sb, all queries; left half masked LO)
        yt_sb = [None] * HPG
        ytn_sb = []
        for pr in range(2):
            t = ytpool.tile([128, T], f16, name=f"ytn{pr}", tag=f"ytn{pr}")
            ytn_sb.append(t)
        rt_sb = [
            const.tile([16, 128], f32, name=f"rt{p}", tag=f"rt{p}")
            for p in range(2)
        ]
        rtf_sb = [
            const.tile([16, 128], f16, name=f"rtf{p}", tag=f"rtf{p}")
            for p in range(2)
        ]
        rrow_sb = [
            const.tile([1, T], f16, name=f"rrow{h}", tag=f"rrow{h}")
            for h in range(HPG)
        ]

        for h in range(HPG):
            yt_sb[h] = ytpool.tile([65, T], f32, name=f"yt{h}", tag=f"yt{h}")

        def emit_C(pr, half, fill=None):
            heads = (2 * pr, 2 * pr + 1)
            for sb in range(half * 4, half * 4 + 4):
                if fill is not None:
                    fill(sb - half * 4)
                q0 = sb * 256
                st = ps_st.tile([128, 1536], f32, name="st", tag="st")
                # score matmuls, head pair interleaved (disjoint PE rows).
                # start=True clears has_written for the WHOLE PSUM bank, so
                # it must be set exactly on the first MM touching each bank.
                started = set()

                def st_mm(c0, nn, lhsT, rhs, stop=False):
                    bank = c0 // 512
                    nc.tensor.matmul(
                        st[:, c0:c0 + nn], lhsT, rhs,
                        start=(bank not in started), stop=stop,
                        skip_group_check=True,
                    )
                    started.add(bank)

                steps = []          # (col_off, N, tkb, q_off, q_n)
                if sb > 0:
                    steps.append((0, 256, 2 * sb - 1, 0, 256))
                    steps.append((256, 128, 2 * sb - 2, 0, 128))
                steps.append((384, 128, 2 * sb + 1, 128, 128))
                steps.append((512, 256, 2 * sb, 0, 256))
                for co, nn, tkb, qo, qn in steps:
                    for hi, h in enumerate(heads):
                        st_mm(
                            hi * 768 + co, nn,
                            kT_h(h)[:, tkb * 128:(tkb + 1) * 128],
                            qT_h(h)[:, q0 + qo:q0 + qo + qn],
                        )
                # additive -30000 band/causal masks via identity matmuls;
                # grouped by lhsT so the PE loads each triangle once
                if sb > 0:
                    st_mm(128, 256, mask_up, ident2)
                    st_mm(896, 128, mask_up, ident)
                    st_mm(1024, 128, mask_up, ident)
                st_mm(384, 128, mask_lo, ident)
                st_mm(512, 128, mask_lo, ident)
                st_mm(1152, 256, mask_lo, ident2, stop=True)
                # exp over the whole pair in one activation
                expst = expst_pool.tile([128, 1536], f16, name="expst",
                                        tag="expst")
                if sb > 0:
                    nc.scalar.activation(
                        expst[:], st[:],
                        mybir.ActivationFunctionType.Exp, scale=0.125,
                    )
                else:
                    for hi in range(2):
                        nc.scalar.activation(
                            expst[:, hi * 768 + 384:hi * 768 + 768],
                            st[:, hi * 768 + 384:hi * 768 + 768],
                            mybir.ActivationFunctionType.Exp, scale=0.125,
                        )
                # att @ [v|1]
                for hi, h in enumerate(heads):
                    yts = ps_bk.tile([65, 256], f32, name="yts", tag="bank")
                    av = []         # (col_off, N, tkb, out_off)
                    if sb > 0:
                        av.append((0, 256, 2 * sb - 1, 0))
                    av.append((512, 256, 2 * sb, 0))
                    if sb > 0:
                        av.append((256, 128, 2 * sb - 2, 0))
                    av.append((384, 128, 2 * sb + 1, 128))
                    for j, (co, nn, tkb, oo) in enumerate(av):
                        nc.tensor.matmul(
                            yts[:, oo:oo + nn],
                            vplus_sb[tkb][:, h, :],
                            expst[:, hi * 768 + co:hi * 768 + co + nn],
                            start=(j == 0),
                            stop=(j == len(av) - 1),
                        )
                    nc.vector.tensor_copy(
                        yt_sb[h][:, q0:q0 + 256], yts[:]
                    )

        def emit_D_recip(pr, half):
            heads = (2 * pr, 2 * pr + 1)
            hT = T // 2
            rt = rt_sb[pr]
            for h in heads:
                r0 = (h % 2) * 8
                q = nc.sync if h % 2 == 0 else nc.scalar
                q.dma_start(rt[r0:r0 + 8, :],
                            yt_sb[h][64:65, half * hT:(half + 1) * hT])
            with nc.allow_low_precision(reason="softmax denom reciprocal"):
                nc.vector.reciprocal(rtf_sb[pr][0:16, :], rt[0:16, :])
            for h in heads:
                r0 = (h % 2) * 8
                q = nc.sync if h % 2 == 0 else nc.scalar
                q.dma_start(rrow_sb[h][0:1, half * hT:(half + 1) * hT],
                            rtf_sb[pr][r0:r0 + 8, :])

        def emit_D_norm(pr, t4):
            heads = (2 * pr, 2 * pr + 1)
            bc = ps_bk.tile([128, 512], f32, name="bc", tag="bank")
            for h in heads:
                p0 = (h % 2) * 64
                nc.tensor.matmul(
                    bc[p0:p0 + 64, :],
                    ones_row,
                    rrow_sb[h][0:1, t4 * 512:(t4 + 1) * 512],
                    start=True,
                    stop=(h == heads[1]),
                    skip_group_check=True,
                )
            for h in heads:
                p0 = (h % 2) * 64
                nc.vector.tensor_mul(
                    ytn_sb[pr][p0:p0 + 64, t4 * 512:(t4 + 1) * 512],
                    yt_sb[h][0:64, t4 * 512:(t4 + 1) * 512],
                    bc[p0:p0 + 64, :],
                )

        def emit_E(tb):
            for nh in range(2):
                ps = ps_bk.tile([128, 512], f32, name="psE", tag="bank")
                for pr in range(2):
                    nc.tensor.matmul(
                        ps[:],
                        ytn_sb[pr][:, tb * 128:(tb + 1) * 128],
                        wp_sb[pr][:, nh * 512:(nh + 1) * 512],
                        start=(pr == 0),
                        stop=(pr == 1),
                    )
                ob = outsb_pool.tile([128, 512], f16, name="outsb", tag="outsb")
                if (tb + nh) % 2 == 0:
                    nc.scalar.copy(ob[:], ps[:])
                else:
                    nc.vector.tensor_copy(ob[:], ps[:])
                qo = nc.sync if (tb * 2 + nh) % 2 == 0 else nc.scalar
                qo.dma_start(
                    out[tb * 128:(tb + 1) * 128, nh * 512:(nh + 1) * 512], ob[:]
                )

        # half-pipelined: half-0's recip/norm overlap half-0 pair-1 and
        # half-1 attention; half-0's projection interleaves into half-1's
        # C emission so its copies/DMAs drain during half-1 compute.
        emit_C(0, 0)
        emit_D_recip(0, 0)
        emit_C(1, 0)
        for t4 in (0, 1):
            emit_D_norm(0, t4)
        emit_D_recip(1, 0)
        for t4 in (0, 1):
            emit_D_norm(1, t4)
        emit_C(0, 1, fill=lambda i: emit_E(i))
        emit_D_recip(0, 1)
        emit_C(1, 1, fill=lambda i: emit_E(4 + i))
        for t4 in (2, 3):
            emit_D_norm(0, t4)
        emit_D_recip(1, 1)
        for t4 in (2, 3):
            emit_D_norm(1, t4)
            for tb in range(4 * t4, 4 * t4 + 4):
                emit_E(tb)

        if debug is not None:
            d_qkT, d_vplus, d_yt, d_rrow, d_ytn = debug
            for m in range(4):
                nc.gpsimd.dma_start(d_qkT[m * 128:(m + 1) * 128, :],
                                    qkT_sb[m][:])
            for tb in range(TB):
                nc.gpsimd.dma_start(
                    d_vplus[tb * 128:(tb + 1) * 128, :],
                    vplus_sb[tb][:].rearrange("p h d -> p (h d)"),
                )
            for h in range(HPG):
                nc.gpsimd.dma_start(d_yt[h * 65:(h + 1) * 65, :], yt_sb[h][:])
                nc.gpsimd.dma_start(d_rrow[h:h + 1, :], rrow_sb[h][:])
            for pr in range(2):
                nc.gpsimd.dma_start(d_ytn[pr * 128:(pr + 1) * 128, :],
                                    ytn_sb[pr][:])


def build_program():
    key = "v12"
    if key in _PROGRAM_CACHE:
        return _PROGRAM_CACHE[key]
    import concourse.bacc as bacc
    import concourse.mybir as mybir
    import concourse.tile as tile

    f16 = mybir.dt.float16
    nc = bacc.Bacc("TRN2", target_bir_lowering=False, debug=False,
                   num_devices=N_CORES)
    xT = nc.dram_tensor("xT", [C, T], f16, kind="ExternalInput").ap()
    wqk = nc.dram_tensor("wqk", [C, 2 * GC], f16, kind="ExternalInput").ap()
    wv = nc.dram_tensor("wv", [C, GC], f16, kind="ExternalInput").ap()
    wp = nc.dram_tensor("wp", [GC, C], f16, kind="ExternalInput").ap()
    aux = nc.dram_tensor("aux", [128, AUX_W], f16, kind="ExternalInput").ap()
    out = nc.dram_tensor("out", [T, C], f16, kind="ExternalOutput").ap()
    with tile.TileContext(nc) as tc:
        _emit(tc, nc, xT, wqk, wv, wp, aux, out)
    nc.compile()
    _PROGRAM_CACHE[key] = nc
    return nc


def _make_aux():
    aux = np.zeros((128, AUX_W), dtype=np.float32)
    aux[:, 0:68] = 1.0
    i = np.arange(128)
    aux[i, AUX_I + i] = 1.0
    aux[i, AUX_I + 128 + i] = 1.0
    r = i[:, None]
    c = i[None, :]
    aux[:, AUX_UP:AUX_UP + 128] = np.where(c < r, NEG, 0.0)
    aux[:, AUX_LO:AUX_LO + 128] = np.where(c > r, NEG, 0.0)
    return aux.astype(np.float16)


def make_in_maps(x, W_attn, W_proj):
    x = np.asarray(x, dtype=np.float32)
    W_attn = np.asarray(W_attn, dtype=np.float32)
    W_proj = np.asarray(W_proj, dtype=np.float32)
    cast = lambda a: np.ascontiguousarray(a, dtype=np.float16)
    xTs = [cast(x[b].T) for b in range(B)]
    aux = _make_aux()
    in_maps = []
    for c in range(N_CORES):
        b, g = divmod(c, G)
        q_cols = W_attn[:, g * GC:(g + 1) * GC]
        k_cols = W_attn[:, C + g * GC:C + (g + 1) * GC]
        v_cols = W_attn[:, 2 * C + g * GC:2 * C + (g + 1) * GC]
        in_maps.append({
            "xT": xTs[b],
            "wqk": cast(np.concatenate([q_cols, k_cols], axis=1)),
            "wv": cast(v_cols),
            "wp": cast(W_proj[g * GC:(g + 1) * GC, :]),
            "aux": aux,
        })
    return in_maps


def gather(results):
    out = np.zeros((B, T, C), dtype=np.float32)
    for c, res in enumerate(results):
        b = c // G
        out[b] += np.asarray(res["out"], dtype=np.float32)
    return out


def kernel(x, W_attn, W_proj, dtype=None, trace=False):
    from concourse import bass_utils

    nc = build_program()
    in_maps = make_in_maps(x, W_attn, W_proj)
    r = bass_utils.run_bass_kernel_spmd(
        nc, in_maps, core_ids=list(range(N_CORES)), trace=trace
    )
    out = gather(r.results)
    if trace:
        kernel.last_results = r
    return out


# revision 17
# speedup vs baseline: 1.0813x; 1.0813x over previous
"""Banded causal self-attention (sparse_attention) for 8 trn2 NeuronCores.

Sharding: tensor-parallel over head groups (4 groups x 4 heads of dim 64)
x data-parallel over batch (2). Core c handles batch c//4, head group c%4.
Each core computes a partial output projection; the host sums the 4 group
partials per batch (partials are written as fp16 to halve output DMA).

v12 schedule (all matmul operands fp16, PSUM f32):
  A: qkT[512, T] = W_qk.T @ x.T   -- k-outer, 8 concurrent PSUM chains so
     matmuls stream while the xT chunks are still DMA-ing in (keeps the PE
     HAM clock warm from ~2us onward).
  B: v[T, 256] = x @ W_v (+ ones column for the softmax denominator).
  C: per head pair (row bases 0/64 of shared tiles so score matmuls of a
     pair run on disjoint PE row groups): banded scores for a 256-query
     superblock live in one [128, 1536] PSUM tile (3 banks, region layout
     [b|a|d|c] per head chosen so no matmul output crosses a bank).
     The causal/band masks are applied as additive -30000 matmuls against
     precomputed triangular lhsT tiles with identity rhs (no gpsimd/DVE in
     the chain), then ONE exp activation per (sb, pair) covers both heads.
     att@v accumulates [v|1] so row 64 of yts is the softmax denominator.
  D: reciprocal of the denominators via a [32,128] bounce tile (DVE recip
     is 8 cyc/elem; never run it on a [1, T] row), broadcast via K=1
     matmuls pair-packed into [128, 512] PSUM, normalize on DVE.
  E: out = y_norm @ W_p, K=128 per pair, accumulated over the 2 pairs;
     fp16 partials stream to DRAM per 128-row block.
"""

import numpy as np

B, T, C = 2, 2048, 1024
N_HEAD = 16
MEMORY = 256
D = 64           # head dim
G = 4            # head groups (tensor parallel)
HPG = 4          # heads per group
GC = HPG * D     # 256 columns per group
N_CORES = 8
TB = T // 128    # 16 row blocks
SB = T // 256    # 8 query super-blocks
NEG = -30000.0   # additive mask; exp(0.125 * -30000) == 0 in f32

# aux tensor column layout
AUX_ONES = 0          # [:, 0:68] ones (vplus ones col + bc ones row)
AUX_I = 68            # [:, 68:196] identity, [:, 196:324] identity (I2)
AUX_UP = 324          # [:, 324:452] lhsT for "keep p>=f" mask (strict lower tri = NEG)
AUX_LO = 452          # [:, 452:580] lhsT for "keep p<=f" mask (strict upper tri = NEG)
AUX_W = 580

_PROGRAM_CACHE = {}


def _emit(tc, nc, xT, wqk, wv, wp, aux, out, debug=None):
    import concourse.mybir as mybir

    f32 = mybir.dt.float32
    f16 = mybir.dt.float16

    from contextlib import ExitStack

    ctx = ExitStack()
    with ctx:
        const = ctx.enter_context(tc.tile_pool(name="const", bufs=1))
        wpool = ctx.enter_context(tc.tile_pool(name="wpool", bufs=1))
        xpool = ctx.enter_context(tc.tile_pool(name="xpool", bufs=1))
        qkt_pool = ctx.enter_context(tc.tile_pool(name="qkt", bufs=1))
        vplus_pool = ctx.enter_context(tc.tile_pool(name="vplus", bufs=1))
        expst_pool = ctx.enter_context(tc.tile_pool(name="expst", bufs=3))
        ytpool = ctx.enter_context(tc.tile_pool(name="yt", bufs=1))
        outsb_pool = ctx.enter_context(tc.tile_pool(name="outsb", bufs=3))
        # PSUM: "st" 2 x 3 banks + "bank" 2 x 1 bank = 8 banks
        ps_st = ctx.enter_context(tc.tile_pool(name="ps_st", bufs=2, space="PSUM"))
        ps_bk = ctx.enter_context(tc.tile_pool(name="ps_bk", bufs=2, space="PSUM"))

        # ---- aux constants (ones / identity / mask triangles) ----
        aux_sb = const.tile([128, AUX_W], f16, name="aux", tag="aux")
        nc.gpsimd.dma_start(aux_sb[:], aux[:])
        ones_row = aux_sb[0:1, 0:64]
        ident = aux_sb[:, AUX_I:AUX_I + 128]
        ident2 = aux_sb[:, AUX_I:AUX_I + 256]
        mask_up = aux_sb[:, AUX_UP:AUX_UP + 128]
        mask_lo = aux_sb[:, AUX_LO:AUX_LO + 128]

        # ---- HAM pre-warm: dummy matmuls on zeros while input DMA runs ----
        # The PE clock sits at 1.2 GHz until ~3.4us of sustained activity;
        # burn the DMA prefix warming it so phase A runs at 2.4 GHz.
        wz = const.tile([128, 512], f16, name="warmz", tag="warmz")
        nc.vector.memzero(wz[:])
        warm_ps = ps_bk.tile([128, 512], f32, name="psW", tag="bank")
        for i in range(18):
            nc.tensor.matmul(warm_ps[:], wz[:, 0:128], wz[:],
                             start=True, stop=True, skip_group_check=True)

        # ---- input loads: xT + wqk chunk-interleaved (phase A streams) ----
        xT_sb, wqk_sb, wv_sb = [], [], []
        for k in range(8):
            qa = nc.sync if k % 2 == 0 else nc.scalar
            qb = nc.scalar if k % 2 == 0 else nc.sync
            t = xpool.tile([128, T], f16, name=f"xT{k}", tag=f"xT{k}")
            qa.dma_start(t[:], xT[k * 128:(k + 1) * 128, :])
            xT_sb.append(t)
            t = wpool.tile([128, 2 * GC], f16, name=f"wqk{k}", tag=f"wqk{k}")
            qb.dma_start(t[:], wqk[k * 128:(k + 1) * 128, :])
            wqk_sb.append(t)

        # ---- phase A: qkT[512, T], k-outer with 8 concurrent chains ----
        qkT_sb = []
        for m in range(4):
            t = qkt_pool.tile([128, T], f16, name=f"qkT{m}", tag=f"qkT{m}")
            qkT_sb.append(t)

        def emit_A(ms):
            # 8 chains: (m in ms) x (t4 in 0..3); chain (m, t4) lives in a
            # fixed 512-col region of an st slot (3 chains) or a bank slot.
            st0 = ps_st.tile([128, 1536], f32, name="psA0", tag="st")
            st1 = ps_st.tile([128, 1536], f32, name="psA1", tag="st")
            bk0 = ps_bk.tile([128, 512], f32, name="psA2", tag="bank")
            bk1 = ps_bk.tile([128, 512], f32, name="psA3", tag="bank")
            regions = {}
            for mi, m in enumerate(ms):
                stt, bkt = (st0, bk0) if mi == 0 else (st1, bk1)
                for t4 in range(4):
                    if t4 < 3:
                        regions[(m, t4)] = stt[:, t4 * 512:(t4 + 1) * 512]
                    else:
                        regions[(m, t4)] = bkt[:]
            for k in range(8):
                for m in ms:
                    for t4 in range(4):
                        nc.tensor.matmul(
                            regions[(m, t4)],
                            wqk_sb[k][:, m * 128:(m + 1) * 128],
                            xT_sb[k][:, t4 * 512:(t4 + 1) * 512],
                            start=(k == 0),
                            stop=(k == 7),
                        )
            for i, ((m, t4), reg) in enumerate(regions.items()):
                eng = nc.scalar.copy if i % 2 == 0 else nc.vector.tensor_copy
                eng(qkT_sb[m][:, t4 * 512:(t4 + 1) * 512], reg)

        emit_A((0, 2))

        # weights for later phases land while A part 2 computes
        for k in range(8):
            qb = nc.scalar if k % 2 == 0 else nc.sync
            t = wpool.tile([128, GC], f16, name=f"wv{k}", tag=f"wv{k}")
            qb.dma_start(t[:], wv[k * 128:(k + 1) * 128, :])
            wv_sb.append(t)
        wp_sb = []
        for pr in range(2):
            t = wpool.tile([128, C], f16, name=f"wp{pr}", tag=f"wp{pr}")
            nc.gpsimd.dma_start(t[:], wp[pr * 128:(pr + 1) * 128, :])
            wp_sb.append(t)

        emit_A((1, 3))

        # ---- phase B: v[T, 256] (+ ones col) ----
        vplus_sb = []
        for tb in range(TB):
            ps = ps_bk.tile([128, GC], f32, name="psB", tag="bank")
            for k in range(8):
                nc.tensor.matmul(
                    ps[:],
                    xT_sb[k][:, tb * 128:(tb + 1) * 128],
                    wv_sb[k][:],
                    start=(k == 0),
                    stop=(k == 7),
                )
            vp = vplus_pool.tile([128, HPG, D + 1], f16, name=f"vplus{tb}",
                                 tag=f"vplus{tb}")
            eng = nc.scalar.copy if tb % 2 == 0 else nc.vector.tensor_copy
            eng(vp[:, :, 0:D], ps[:].rearrange("p (h d) -> p h d", h=HPG))
            q = nc.sync if tb % 2 == 0 else nc.scalar
            q.dma_start(
                vp[:, :, D:D + 1],
                aux[:, 0:HPG].rearrange("p (h o) -> p h o", o=1),
            )
            vplus_sb.append(vp)

        # per-head views into qkT: q rows = h*64.., k rows = 256 + h*64..
        def qT_h(h):
            return qkT_sb[h // 2][(h % 2) * 64:(h % 2) * 64 + 64, :]

        def kT_h(h):
            return qkT_sb[2 + h // 2][(h % 2) * 64:(h % 2) * 64 + 64, :]

        # region layout per head within the [128, 1536] score tile:
        #   b: +0:256 (tkb=2sb-1, all 256 queries; right half masked UP)
        #   a: +256:384 (tkb=2sb-2, queries 0:128; masked UP)
        #   d: +384:512 (tkb=2sb+1, queries 128:256; masked LO)
        #   c: +512:768 (tkb=2sb, all queries; left half masked LO)
        yt_sb = [None] * HPG
        ytn_sb = []
        for pr in range(2):
            t = ytpool.tile([128, T], f16, name=f"ytn{pr}", tag=f"ytn{pr}")
            ytn_sb.append(t)
        rt_sb = [
            const.tile([16, 128], f32, name=f"rt{p}", tag=f"rt{p}")
            for p in range(2)
        ]
        rtf_sb = [
            const.tile([16, 128], f16, name=f"rtf{p}", tag=f"rtf{p}")
            for p in range(2)
        ]
        rrow_sb = [
            const.tile([1, T], f16, name=f"rrow{h}", tag=f"rrow{h}")
            for h in range(HPG)
        ]

        for h in range(HPG):
            yt_sb[h] = ytpool.tile([65, T], f32, name=f"yt{h}", tag=f"yt{h}")

        def emit_C(pr, half, fill=None):
            heads = (2 * pr, 2 * pr + 1)
            for sb in range(half * 4, half * 4 + 4):
                if fill is not None:
                    fill(sb - half * 4)
                q0 = sb * 256
                st = ps_st.tile([128, 1536], f32, name="st", tag="st")
                # score matmuls, head pair interleaved (disjoint PE rows).
                # start=True clears has_written for the WHOLE PSUM bank, so
                # it must be set exactly on the first MM touching each bank.
                started = set()

                def st_mm(c0, nn, lhsT, rhs, stop=False):
                    bank = c0 // 512
                    nc.tensor.matmul(
                        st[:, c0:c0 + nn], lhsT, rhs,
                        start=(bank not in started), stop=stop,
                        skip_group_check=True,
                    )
                    started.add(bank)

                steps = []          # (col_off, N, tkb, q_off, q_n)
                if sb > 0:
                    steps.append((0, 256, 2 * sb - 1, 0, 256))
                    steps.append((256, 128, 2 * sb - 2, 0, 128))
                steps.append((384, 128, 2 * sb + 1, 128, 128))
                steps.append((512, 256, 2 * sb, 0, 256))
                for co, nn, tkb, qo, qn in steps:
                    for hi, h in enumerate(heads):
                        st_mm(
                            hi * 768 + co, nn,
                            kT_h(h)[:, tkb * 128:(tkb + 1) * 128],
                            qT_h(h)[:, q0 + qo:q0 + qo + qn],
                        )
                # additive -30000 band/causal masks via identity matmuls;
                # grouped by lhsT so the PE loads each triangle once
                if sb > 0:
                    st_mm(128, 256, mask_up, ident2)
                    st_mm(896, 128, mask_up, ident)
                    st_mm(1024, 128, mask_up, ident)
                st_mm(384, 128, mask_lo, ident)
                st_mm(512, 128, mask_lo, ident)
                st_mm(1152, 256, mask_lo, ident2, stop=True)
                # exp over the whole pair in one activation
                expst = expst_pool.tile([128, 1536], f16, name="expst",
                                        tag="expst")
                if sb > 0:
                    nc.scalar.activation(
                        expst[:], st[:],
                        mybir.ActivationFunctionType.Exp, scale=0.125,
                    )
                else:
                    for hi in range(2):
                        nc.scalar.activation(
                            expst[:, hi * 768 + 384:hi * 768 + 768],
                            st[:, hi * 768 + 384:hi * 768 + 768],
                            mybir.ActivationFunctionType.Exp, scale=0.125,
                        )
                # att @ [v|1]
                for hi, h in enumerate(heads):
                    yts = ps_bk.tile([65, 256], f32, name="yts", tag="bank")
                    av = []         # (col_off, N, tkb, out_off)
                    if sb > 0:
                        av.append((0, 256, 2 * sb - 1, 0))
                    av.append((512, 256, 2 * sb, 0))
                    if sb > 0:
                        av.append((256, 128, 2 * sb - 2, 0))
                    av.append((384, 128, 2 * sb + 1, 128))
                    for j, (co, nn, tkb, oo) in enumerate(av):
                        nc.tensor.matmul(
                            yts[:, oo:oo + nn],
                            vplus_sb[tkb][:, h, :],
                            expst[:, hi * 768 + co:hi * 768 + co + nn],
                            start=(j == 0),
                            stop=(j == len(av) - 1),
                        )
                    nc.vector.tensor_copy(
                        yt_sb[h][:, q0:q0 + 256], yts[:]
                    )

        def emit_D_recip(pr, half):
            heads = (2 * pr, 2 * pr + 1)
            hT = T // 2
            rt = rt_sb[pr]
            for h in heads:
                r0 = (h % 2) * 8
                q = nc.sync if h % 2 == 0 else nc.scalar
                q.dma_start(rt[r0:r0 + 8, :],
                            yt_sb[h][64:65, half * hT:(half + 1) * hT])
            with nc.allow_low_precision(reason="softmax denom reciprocal"):
                nc.vector.reciprocal(rtf_sb[pr][0:16, :], rt[0:16, :])
            for h in heads:
                r0 = (h % 2) * 8
                q = nc.sync if h % 2 == 0 else nc.scalar
                q.dma_start(rrow_sb[h][0:1, half * hT:(half + 1) * hT],
                            rtf_sb[pr][r0:r0 + 8, :])

        def emit_D_norm(pr, t4):
            heads = (2 * pr, 2 * pr + 1)
            bc = ps_bk.tile([128, 512], f32, name="bc", tag="bank")
            for h in heads:
                p0 = (h % 2) * 64
                nc.tensor.matmul(
                    bc[p0:p0 + 64, :],
                    ones_row,
                    rrow_sb[h][0:1, t4 * 512:(t4 + 1) * 512],
                    start=True,
                    stop=(h == heads[1]),
                    skip_group_check=True,
                )
            for h in heads:
                p0 = (h % 2) * 64
                nc.vector.tensor_mul(
                    ytn_sb[pr][p0:p0 + 64, t4 * 512:(t4 + 1) * 512],
                    yt_sb[h][0:64, t4 * 512:(t4 + 1) * 512],
                    bc[p0:p0 + 64, :],
                )

        def emit_E(tb):
            for nh in range(2):
                ps = ps_bk.tile([128, 512], f32, name="psE", tag="bank")
                for pr in range(2):
                    nc.tensor.matmul(
                        ps[:],
                        ytn_sb[pr][:, tb * 128:(tb + 1) * 128],
                        wp_sb[pr][:, nh * 512:(nh + 1) * 512],
                        start=(pr == 0),
                        stop=(pr == 1),
                    )
                ob = outsb_pool.tile([128, 512], f16, name="outsb", tag="outsb")
                if (tb + nh) % 2 == 0:
                    nc.scalar.copy(ob[:], ps[:])
                else:
                    nc.vector.tensor_copy(ob[:], ps[:])
                qo = nc.sync if (tb * 2 + nh) % 2 == 0 else nc.scalar
                qo.dma_start(
                    out[tb * 128:(tb + 1) * 128, nh * 512:(nh + 1) * 512], ob[:]
                )

        # Half-pipelined: each half's reciprocal + normalize is emitted as a
        # light fill inside the NEXT C phase (one bc matmul + 2 DVE muls per
        # filled superblock), so by the time phase E starts everything except
        # the last pair-half's normalize is already done.
        def norm_fill(jobs):
            return lambda i: [emit_D_norm(pr, t4) for pr, t4 in jobs.get(i, [])]

        emit_C(0, 0)
        emit_D_recip(0, 0)
        emit_C(1, 0, fill=norm_fill({2: [(0, 0)], 3: [(0, 1)]}))
        emit_D_recip(1, 0)
        emit_C(0, 1, fill=norm_fill({2: [(1, 0)], 3: [(1, 1)]}))
        emit_D_recip(0, 1)
        emit_C(1, 1, fill=norm_fill({2: [(0, 2)], 3: [(0, 3)]}))
        emit_D_recip(1, 1)
        emit_E(0)
        emit_E(1)
        emit_D_norm(1, 2)
        emit_E(2)
        emit_E(3)
        emit_D_norm(1, 3)
        for tb in range(4, 16):
            emit_E(tb)

        if debug is not None:
            d_qkT, d_vplus, d_yt, d_rrow, d_ytn = debug
            for m in range(4):
                nc.gpsimd.dma_start(d_qkT[m * 128:(m + 1) * 128, :],
                                    qkT_sb[m][:])
            for tb in range(TB):
                nc.gpsimd.dma_start(
                    d_vplus[tb * 128:(tb + 1) * 128, :],
                    vplus_sb[tb][:].rearrange("p h d -> p (h d)"),
                )
            for h in range(HPG):
                nc.gpsimd.dma_start(d_yt[h * 65:(h + 1) * 65, :], yt_sb[h][:])
                nc.gpsimd.dma_start(d_rrow[h:h + 1, :], rrow_sb[h][:])
            for pr in range(2):
                nc.gpsimd.dma_start(d_ytn[pr * 128:(pr + 1) * 128, :],
                                    ytn_sb[pr][:])


def build_program():
    key = "v12"
    if key in _PROGRAM_CACHE:
        return _PROGRAM_CACHE[key]
    import concourse.bacc as bacc
    import concourse.mybir as mybir
    import concourse.tile as tile

    f16 = mybir.dt.float16
    nc = bacc.Bacc("TRN2", target_bir_lowering=False, debug=False,
                   num_devices=N_CORES)
    xT = nc.dram_tensor("xT", [C, T], f16, kind="ExternalInput").ap()
    wqk = nc.dram_tensor("wqk", [C, 2 * GC], f16, kind="ExternalInput").ap()
    wv = nc.dram_tensor("wv", [C, GC], f16, kind="ExternalInput").ap()
    wp = nc.dram_tensor("wp", [GC, C], f16, kind="ExternalInput").ap()
    aux = nc.dram_tensor("aux", [128, AUX_W], f16, kind="ExternalInput").ap()
    out = nc.dram_tensor("out", [T, C], f16, kind="ExternalOutput").ap()
    with tile.TileContext(nc) as tc:
        _emit(tc, nc, xT, wqk, wv, wp, aux, out)
    nc.compile()
    _PROGRAM_CACHE[key] = nc
    return nc


def _make_aux():
    aux = np.zeros((128, AUX_W), dtype=np.float32)
    aux[:, 0:68] = 1.0
    i = np.arange(128)
    aux[i, AUX_I + i] = 1.0
    aux[i, AUX_I + 128 + i] = 1.0
    r = i[:, None]
    c = i[None, :]
    aux[:, AUX_UP:AUX_UP + 128] = np.where(c < r, NEG, 0.0)
    aux[:, AUX_LO:AUX_LO + 128] = np.where(c > r, NEG, 0.0)
    return aux.astype(np.float16)


def make_in_maps(x, W_attn, W_proj):
    x = np.asarray(x, dtype=np.float32)
    W_attn = np.asarray(W_attn, dtype=np.float32)
    W_proj = np.asarray(W_proj, dtype=np.float32)
    cast = lambda a: np.ascontiguousarray(a, dtype=np.float16)
    xTs = [cast(x[b].T) for b in range(B)]
    aux = _make_aux()
    in_maps = []
    for c in range(N_CORES):
        b, g = divmod(c, G)
        q_cols = W_attn[:, g * GC:(g + 1) * GC]
        k_cols = W_attn[:, C + g * GC:C + (g + 1) * GC]
        v_cols = W_attn[:, 2 * C + g * GC:2 * C + (g + 1) * GC]
        in_maps.append({
            "xT": xTs[b],
            "wqk": cast(np.concatenate([q_cols, k_cols], axis=1)),
            "wv": cast(v_cols),
            "wp": cast(W_proj[g * GC:(g + 1) * GC, :]),
            "aux": aux,
        })
    return in_maps


def gather(results):
    out = np.zeros((B, T, C), dtype=np.float32)
    for c, res in enumerate(results):
        b = c // G
        out[b] += np.asarray(res["out"], dtype=np.float32)
    return out


def kernel(x, W_attn, W_proj, dtype=None, trace=False):
    from concourse import bass_utils

    nc = build_program()
    in_maps = make_in_maps(x, W_attn, W_proj)
    r = bass_utils.run_bass_kernel_spmd(
        nc, in_maps, core_ids=list(range(N_CORES)), trace=trace
    )
    out = gather(r.results)
    if trace:
        kernel.last_results = r
    return out


# revision 20
# speedup vs baseline: 1.1414x; 1.0555x over previous
"""Banded causal self-attention (sparse_attention) for 8 trn2 NeuronCores.

Sharding: tensor-parallel over head groups (4 groups x 4 heads of dim 64)
x data-parallel over batch (2). Core c handles batch c//4, head group c%4.
Each core computes a partial output projection; the host sums the 4 group
partials per batch (partials are written as fp16 to halve output DMA).

v12 schedule (all matmul operands fp16, PSUM f32):
  A: qkT[512, T] = W_qk.T @ x.T   -- k-outer, 8 concurrent PSUM chains so
     matmuls stream while the xT chunks are still DMA-ing in (keeps the PE
     HAM clock warm from ~2us onward).
  B: v[T, 256] = x @ W_v (+ ones column for the softmax denominator).
  C: per head pair (row bases 0/64 of shared tiles so score matmuls of a
     pair run on disjoint PE row groups): banded scores for a 256-query
     superblock live in one [128, 1536] PSUM tile (3 banks, region layout
     [b|a|d|c] per head chosen so no matmul output crosses a bank).
     The causal/band masks are applied as additive -30000 matmuls against
     precomputed triangular lhsT tiles with identity rhs (no gpsimd/DVE in
     the chain), then ONE exp activation per (sb, pair) covers both heads.
     att@v accumulates [v|1] so row 64 of yts is the softmax denominator.
  D: reciprocal of the denominators via a [32,128] bounce tile (DVE recip
     is 8 cyc/elem; never run it on a [1, T] row), broadcast via K=1
     matmuls pair-packed into [128, 512] PSUM, normalize on DVE.
  E: out = y_norm @ W_p, K=128 per pair, accumulated over the 2 pairs;
     fp16 partials stream to DRAM per 128-row block.
"""

import numpy as np

B, T, C = 2, 2048, 1024
N_HEAD = 16
MEMORY = 256
D = 64           # head dim
G = 4            # head groups (tensor parallel)
HPG = 4          # heads per group
GC = HPG * D     # 256 columns per group
N_CORES = 8
TB = T // 128    # 16 row blocks
SB = T // 256    # 8 query super-blocks
NEG = -30000.0   # additive mask; exp(0.125 * -30000) == 0 in f32

# aux tensor column layout
AUX_ONES = 0          # [:, 0:68] ones (vplus ones col + bc ones row)
AUX_I = 68            # [:, 68:196] identity, [:, 196:324] identity (I2)
AUX_UP = 324          # [:, 324:452] lhsT for "keep p>=f" mask (strict lower tri = NEG)
AUX_LO = 452          # [:, 452:580] lhsT for "keep p<=f" mask (strict upper tri = NEG)
AUX_W = 580

_PROGRAM_CACHE = {}


def _emit(tc, nc, xT, wqk, wv, wp, aux, out, debug=None):
    import concourse.mybir as mybir

    f32 = mybir.dt.float32
    f16 = mybir.dt.float16

    from contextlib import ExitStack

    ctx = ExitStack()
    with ctx:
        const = ctx.enter_context(tc.tile_pool(name="const", bufs=1))
        wpool = ctx.enter_context(tc.tile_pool(name="wpool", bufs=1))
        xpool = ctx.enter_context(tc.tile_pool(name="xpool", bufs=1))
        qkt_pool = ctx.enter_context(tc.tile_pool(name="qkt", bufs=1))
        vplus_pool = ctx.enter_context(tc.tile_pool(name="vplus", bufs=1))
        expst_pool = ctx.enter_context(tc.tile_pool(name="expst", bufs=3))
        ytpool = ctx.enter_context(tc.tile_pool(name="yt", bufs=1))
        outsb_pool = ctx.enter_context(tc.tile_pool(name="outsb", bufs=3))
        # PSUM: "st" 2 x 3 banks + "bank" 2 x 1 bank = 8 banks
        ps_st = ctx.enter_context(tc.tile_pool(name="ps_st", bufs=2, space="PSUM"))
        ps_bk = ctx.enter_context(tc.tile_pool(name="ps_bk", bufs=2, space="PSUM"))

        # ---- aux constants (ones / identity / mask triangles) ----
        aux_sb = const.tile([128, AUX_W], f16, name="aux", tag="aux")
        nc.gpsimd.dma_start(aux_sb[:], aux[:])
        ones_row = aux_sb[0:1, 0:64]
        ident = aux_sb[:, AUX_I:AUX_I + 128]
        ident2 = aux_sb[:, AUX_I:AUX_I + 256]
        mask_up = aux_sb[:, AUX_UP:AUX_UP + 128]
        mask_lo = aux_sb[:, AUX_LO:AUX_LO + 128]

        # ---- HAM pre-warm: dummy matmuls on zeros while input DMA runs ----
        # The PE clock sits at 1.2 GHz until ~3.4us of sustained activity;
        # burn the DMA prefix warming it so phase A runs at 2.4 GHz.
        wz = const.tile([128, 512], f16, name="warmz", tag="warmz")
        nc.vector.memzero(wz[:])

        # ---- input loads: xT + wqk chunk-interleaved (phase A streams) ----
        xT_sb, wqk_sb, wv_sb = [], [], []
        for k in range(8):
            qa = nc.sync if k % 2 == 0 else nc.scalar
            qb = nc.scalar if k % 2 == 0 else nc.sync
            t = xpool.tile([128, T], f16, name=f"xT{k}", tag=f"xT{k}")
            qa.dma_start(t[:], xT[k * 128:(k + 1) * 128, :])
            xT_sb.append(t)
            t = wpool.tile([128, 2 * GC], f16, name=f"wqk{k}", tag=f"wqk{k}")
            qb.dma_start(t[:], wqk[k * 128:(k + 1) * 128, :])
            wqk_sb.append(t)

        # ---- phase A: qkT[512, T], k-outer with 8 concurrent chains ----
        qkT_sb = []
        for m in range(4):
            t = qkt_pool.tile([128, T], f16, name=f"qkT{m}", tag=f"qkT{m}")
            qkT_sb.append(t)

        def emit_A(ms, warm=False):
            # 8 chains: (m in ms) x (t4 in 0..3); chain (m, t4) lives in a
            # fixed 512-col region of an st slot (3 chains) or a bank slot.
            st0 = ps_st.tile([128, 1536], f32, name="psA0", tag="st")
            st1 = ps_st.tile([128, 1536], f32, name="psA1", tag="st")
            bk0 = ps_bk.tile([128, 512], f32, name="psA2", tag="bank")
            bk1 = ps_bk.tile([128, 512], f32, name="psA3", tag="bank")
            if warm:
                # HAM pre-warm: dummy matmuls on zeros while the input DMA
                # streams in (PE clock sits at 1.2 GHz until ~3.4us of
                # sustained activity). Output lands in an A-chain region
                # that the real chain re-initializes via start=True.
                for i in range(14):
                    nc.tensor.matmul(st0[:, 0:512], wz[:, 0:128], wz[:],
                                     start=True, stop=True,
                                     skip_group_check=True)
            regions = {}
            for mi, m in enumerate(ms):
                stt, bkt = (st0, bk0) if mi == 0 else (st1, bk1)
                for t4 in range(4):
                    if t4 < 3:
                        regions[(m, t4)] = stt[:, t4 * 512:(t4 + 1) * 512]
                    else:
                        regions[(m, t4)] = bkt[:]
            for k in range(8):
                for m in ms:
                    for t4 in range(4):
                        nc.tensor.matmul(
                            regions[(m, t4)],
                            wqk_sb[k][:, m * 128:(m + 1) * 128],
                            xT_sb[k][:, t4 * 512:(t4 + 1) * 512],
                            start=(k == 0),
                            stop=(k == 7),
                        )
            for i, ((m, t4), reg) in enumerate(regions.items()):
                eng = nc.scalar.copy if i % 2 == 0 else nc.vector.tensor_copy
                eng(qkT_sb[m][:, t4 * 512:(t4 + 1) * 512], reg)

        emit_A((0, 2), warm=True)

        # weights for later phases land while A part 2 computes
        for k in range(8):
            qb = nc.scalar if k % 2 == 0 else nc.sync
            t = wpool.tile([128, GC], f16, name=f"wv{k}", tag=f"wv{k}")
            qb.dma_start(t[:], wv[k * 128:(k + 1) * 128, :])
            wv_sb.append(t)
        wp_sb = []
        for pr in range(2):
            t = wpool.tile([128, C], f16, name=f"wp{pr}", tag=f"wp{pr}")
            nc.gpsimd.dma_start(t[:], wp[pr * 128:(pr + 1) * 128, :])
            wp_sb.append(t)

        emit_A((1, 3))

        # ---- phase B: v[T, 256] (+ ones col) ----
        vplus_sb = []
        for tb in range(TB):
            ps = ps_bk.tile([128, GC], f32, name="psB", tag="bank")
            for k in range(8):
                nc.tensor.matmul(
                    ps[:],
                    xT_sb[k][:, tb * 128:(tb + 1) * 128],
                    wv_sb[k][:],
                    start=(k == 0),
                    stop=(k == 7),
                )
            vp = vplus_pool.tile([128, HPG, D + 1], f16, name=f"vplus{tb}",
                                 tag=f"vplus{tb}")
            eng = nc.scalar.copy if tb % 2 == 0 else nc.vector.tensor_copy
            eng(vp[:, :, 0:D], ps[:].rearrange("p (h d) -> p h d", h=HPG))
            q = nc.sync if tb % 2 == 0 else nc.scalar
            q.dma_start(
                vp[:, :, D:D + 1],
                aux[:, 0:HPG].rearrange("p (h o) -> p h o", o=1),
            )
            vplus_sb.append(vp)

        # per-head views into qkT: q rows = h*64.., k rows = 256 + h*64..
        def qT_h(h):
            return qkT_sb[h // 2][(h % 2) * 64:(h % 2) * 64 + 64, :]

        def kT_h(h):
            return qkT_sb[2 + h // 2][(h % 2) * 64:(h % 2) * 64 + 64, :]

        # region layout per head within the [128, 1536] score tile:
        #   b: +0:256 (tkb=2sb-1, all 256 queries; right half masked UP)
        #   a: +256:384 (tkb=2sb-2, queries 0:128; masked UP)
        #   d: +384:512 (tkb=2sb+1, queries 128:256; masked LO)
        #   c: +512:768 (tkb=2sb, all queries; left half masked LO)
        yt_sb = [None] * HPG
        ytn_sb = []
        for pr in range(2):
            t = ytpool.tile([128, T], f16, name=f"ytn{pr}", tag=f"ytn{pr}")
            ytn_sb.append(t)
        rt_sb = [
            const.tile([16, 128], f32, name=f"rt{p}", tag=f"rt{p}")
            for p in range(2)
        ]
        rtf_sb = [
            const.tile([16, 128], f16, name=f"rtf{p}", tag=f"rtf{p}")
            for p in range(2)
        ]
        rrow_sb = [
            const.tile([1, T], f16, name=f"rrow{h}", tag=f"rrow{h}")
            for h in range(HPG)
        ]

        for h in range(HPG):
            yt_sb[h] = ytpool.tile([65, T], f32, name=f"yt{h}", tag=f"yt{h}")

        def emit_C(pr, half, fill=None):
            heads = (2 * pr, 2 * pr + 1)
            for sb in range(half * 4, half * 4 + 4):
                if fill is not None:
                    fill(sb - half * 4)
                q0 = sb * 256
                st = ps_st.tile([128, 1536], f32, name="st", tag="st")
                # score matmuls, head pair interleaved (disjoint PE rows).
                # start=True clears has_written for the WHOLE PSUM bank, so
                # it must be set exactly on the first MM touching each bank.
                started = set()

                def st_mm(c0, nn, lhsT, rhs, stop=False):
                    bank = c0 // 512
                    nc.tensor.matmul(
                        st[:, c0:c0 + nn], lhsT, rhs,
                        start=(bank not in started), stop=stop,
                        skip_group_check=True,
                    )
                    started.add(bank)

                steps = []          # (col_off, N, tkb, q_off, q_n)
                if sb > 0:
                    steps.append((0, 256, 2 * sb - 1, 0, 256))
                    steps.append((256, 128, 2 * sb - 2, 0, 128))
                steps.append((384, 128, 2 * sb + 1, 128, 128))
                steps.append((512, 256, 2 * sb, 0, 256))
                for co, nn, tkb, qo, qn in steps:
                    for hi, h in enumerate(heads):
                        st_mm(
                            hi * 768 + co, nn,
                            kT_h(h)[:, tkb * 128:(tkb + 1) * 128],
                            qT_h(h)[:, q0 + qo:q0 + qo + qn],
                        )
                # additive -30000 band/causal masks via identity matmuls;
                # grouped by lhsT so the PE loads each triangle once
                if sb > 0:
                    st_mm(128, 256, mask_up, ident2)
                    st_mm(896, 128, mask_up, ident)
                    st_mm(1024, 128, mask_up, ident)
                st_mm(384, 128, mask_lo, ident)
                st_mm(512, 128, mask_lo, ident)
                st_mm(1152, 256, mask_lo, ident2, stop=True)
                # exp over the whole pair in one activation
                expst = expst_pool.tile([128, 1536], f16, name="expst",
                                        tag="expst")
                if sb > 0:
                    nc.scalar.activation(
                        expst[:], st[:],
                        mybir.ActivationFunctionType.Exp, scale=0.125,
                    )
                else:
                    for hi in range(2):
                        nc.scalar.activation(
                            expst[:, hi * 768 + 384:hi * 768 + 768],
                            st[:, hi * 768 + 384:hi * 768 + 768],
                            mybir.ActivationFunctionType.Exp, scale=0.125,
                        )
                # att @ [v|1]
                for hi, h in enumerate(heads):
                    yts = ps_bk.tile([65, 256], f32, name="yts", tag="bank")
                    av = []         # (col_off, N, tkb, out_off)
                    if sb > 0:
                        av.append((0, 256, 2 * sb - 1, 0))
                    av.append((512, 256, 2 * sb, 0))
                    if sb > 0:
                        av.append((256, 128, 2 * sb - 2, 0))
                    av.append((384, 128, 2 * sb + 1, 128))
                    for j, (co, nn, tkb, oo) in enumerate(av):
                        nc.tensor.matmul(
                            yts[:, oo:oo + nn],
                            vplus_sb[tkb][:, h, :],
                            expst[:, hi * 768 + co:hi * 768 + co + nn],
                            start=(j == 0),
                            stop=(j == len(av) - 1),
                        )
                    nc.vector.tensor_copy(
                        yt_sb[h][:, q0:q0 + 256], yts[:]
                    )

        def emit_D_recip(pr, half):
            heads = (2 * pr, 2 * pr + 1)
            hT = T // 2
            rt = rt_sb[pr]
            for h in heads:
                r0 = (h % 2) * 8
                q = nc.sync if h % 2 == 0 else nc.scalar
                q.dma_start(rt[r0:r0 + 8, :],
                            yt_sb[h][64:65, half * hT:(half + 1) * hT])
            with nc.allow_low_precision(reason="softmax denom reciprocal"):
                nc.vector.reciprocal(rtf_sb[pr][0:16, :], rt[0:16, :])
            for h in heads:
                r0 = (h % 2) * 8
                q = nc.sync if h % 2 == 0 else nc.scalar
                q.dma_start(rrow_sb[h][0:1, half * hT:(half + 1) * hT],
                            rtf_sb[pr][r0:r0 + 8, :])

        def emit_D_norm(pr, t4):
            heads = (2 * pr, 2 * pr + 1)
            bc = ps_bk.tile([128, 512], f32, name="bc", tag="bank")
            for h in heads:
                p0 = (h % 2) * 64
                nc.tensor.matmul(
                    bc[p0:p0 + 64, :],
                    ones_row,
                    rrow_sb[h][0:1, t4 * 512:(t4 + 1) * 512],
                    start=True,
                    stop=(h == heads[1]),
                    skip_group_check=True,
                )
            for h in heads:
                p0 = (h % 2) * 64
                nc.vector.tensor_mul(
                    ytn_sb[pr][p0:p0 + 64, t4 * 512:(t4 + 1) * 512],
                    yt_sb[h][0:64, t4 * 512:(t4 + 1) * 512],
                    bc[p0:p0 + 64, :],
                )

        def emit_E(tb):
            for nh in range(2):
                ps = ps_bk.tile([128, 512], f32, name="psE", tag="bank")
                for pr in range(2):
                    nc.tensor.matmul(
                        ps[:],
                        ytn_sb[pr][:, tb * 128:(tb + 1) * 128],
                        wp_sb[pr][:, nh * 512:(nh + 1) * 512],
                        start=(pr == 0),
                        stop=(pr == 1),
                    )
                ob = outsb_pool.tile([128, 512], f16, name="outsb", tag="outsb")
                if (tb + nh) % 2 == 0:
                    nc.scalar.copy(ob[:], ps[:])
                else:
                    nc.vector.tensor_copy(ob[:], ps[:])
                qo = nc.sync if (tb * 2 + nh) % 2 == 0 else nc.scalar
                qo.dma_start(
                    out[tb * 128:(tb + 1) * 128, nh * 512:(nh + 1) * 512], ob[:]
                )

        # Half-pipelined: each half's reciprocal + normalize is emitted as a
        # light fill inside the NEXT C phase (one bc matmul + 2 DVE muls per
        # filled superblock), so by the time phase E starts everything except
        # the last pair-half's normalize is already done.
        def norm_fill(jobs):
            return lambda i: [emit_D_norm(pr, t4) for pr, t4 in jobs.get(i, [])]

        emit_C(0, 0)
        emit_D_recip(0, 0)
        emit_C(1, 0)
        emit_D_recip(1, 0)
        emit_C(0, 1, fill=norm_fill({1: [(0, 0)], 2: [(0, 1)], 3: [(1, 0)]}))
        emit_D_recip(0, 1)
        emit_C(1, 1, fill=norm_fill({1: [(1, 1)], 2: [(0, 2)], 3: [(0, 3)]}))
        emit_D_recip(1, 1)
        emit_E(0)
        emit_E(1)
        emit_D_norm(1, 2)
        emit_E(2)
        emit_E(3)
        emit_D_norm(1, 3)
        for tb in range(4, 16):
            emit_E(tb)

        if debug is not None:
            d_qkT, d_vplus, d_yt, d_rrow, d_ytn = debug
            for m in range(4):
                nc.gpsimd.dma_start(d_qkT[m * 128:(m + 1) * 128, :],
                                    qkT_sb[m][:])
            for tb in range(TB):
                nc.gpsimd.dma_start(
                    d_vplus[tb * 128:(tb + 1) * 128, :],
                    vplus_sb[tb][:].rearrange("p h d -> p (h d)"),
                )
            for h in range(HPG):
                nc.gpsimd.dma_start(d_yt[h * 65:(h + 1) * 65, :], yt_sb[h][:])
                nc.gpsimd.dma_start(d_rrow[h:h + 1, :], rrow_sb[h][:])
            for pr in range(2):
                nc.gpsimd.dma_start(d_ytn[pr * 128:(pr + 1) * 128, :],
                                    ytn_sb[pr][:])


def build_program():
    key = "v12"
    if key in _PROGRAM_CACHE:
        return _PROGRAM_CACHE[key]
    import concourse.bacc as bacc
    import concourse.mybir as mybir
    import concourse.tile as tile

    f16 = mybir.dt.float16
    nc = bacc.Bacc("TRN2", target_bir_lowering=False, debug=False,
                   num_devices=N_CORES)
    xT = nc.dram_tensor("xT", [C, T], f16, kind="ExternalInput").ap()
    wqk = nc.dram_tensor("wqk", [C, 2 * GC], f16, kind="ExternalInput").ap()
    wv = nc.dram_tensor("wv", [C, GC], f16, kind="ExternalInput").ap()
    wp = nc.dram_tensor("wp", [GC, C], f16, kind="ExternalInput").ap()
    aux = nc.dram_tensor("aux", [128, AUX_W], f16, kind="ExternalInput").ap()
    out = nc.dram_tensor("out", [T, C], f16, kind="ExternalOutput").ap()
    with tile.TileContext(nc) as tc:
        _emit(tc, nc, xT, wqk, wv, wp, aux, out)
    nc.compile()
    _PROGRAM_CACHE[key] = nc
    return nc


def _make_aux():
    aux = np.zeros((128, AUX_W), dtype=np.float32)
    aux[:, 0:68] = 1.0
    i = np.arange(128)
    aux[i, AUX_I + i] = 1.0
    aux[i, AUX_I + 128 + i] = 1.0
    r = i[:, None]
    c = i[None, :]
    aux[:, AUX_UP:AUX_UP + 128] = np.where(c < r, NEG, 0.0)
    aux[:, AUX_LO:AUX_LO + 128] = np.where(c > r, NEG, 0.0)
    return aux.astype(np.float16)


def make_in_maps(x, W_attn, W_proj):
    x = np.asarray(x, dtype=np.float32)
    W_attn = np.asarray(W_attn, dtype=np.float32)
    W_proj = np.asarray(W_proj, dtype=np.float32)
    cast = lambda a: np.ascontiguousarray(a, dtype=np.float16)
    xTs = [cast(x[b].T) for b in range(B)]
    aux = _make_aux()
    in_maps = []
    for c in range(N_CORES):
        b, g = divmod(c, G)
        q_cols = W_attn[:, g * GC:(g + 1) * GC]
        k_cols = W_attn[:, C + g * GC:C + (g + 1) * GC]
        v_cols = W_attn[:, 2 * C + g * GC:2 * C + (g + 1) * GC]
        in_maps.append({
            "xT": xTs[b],
            "wqk": cast(np.concatenate([q_cols, k_cols], axis=1)),
            "wv": cast(v_cols),
            "wp": cast(W_proj[g * GC:(g + 1) * GC, :]),
            "aux": aux,
        })
    return in_maps


def gather(results):
    out = np.zeros((B, T, C), dtype=np.float32)
    for c, res in enumerate(results):
        b = c // G
        out[b] += np.asarray(res["out"], dtype=np.float32)
    return out


def kernel(x, W_attn, W_proj, dtype=None, trace=False):
    from concourse import bass_utils

    nc = build_program()
    in_maps = make_in_maps(x, W_attn, W_proj)
    r = bass_utils.run_bass_kernel_spmd(
        nc, in_maps, core_ids=list(range(N_CORES)), trace=trace
    )
    out = gather(r.results)
    if trace:
        kernel.last_results = r
    return out


# revision 24
# speedup vs baseline: 1.2358x; 1.0828x over previous
"""Banded causal self-attention (sparse_attention) for 8 trn2 NeuronCores.

Sharding: tensor-parallel over head groups (4 groups x 4 heads of dim 64)
x data-parallel over batch (2). Core c handles batch c//4, head group c%4.
Each core computes a partial output projection; the host sums the 4 group
partials per batch (partials are written as fp16 to halve output DMA).

v12 schedule (all matmul operands fp16, PSUM f32):
  A: qkT[512, T] = W_qk.T @ x.T   -- k-outer, 8 concurrent PSUM chains so
     matmuls stream while the xT chunks are still DMA-ing in (keeps the PE
     HAM clock warm from ~2us onward).
  B: v[T, 256] = x @ W_v (+ ones column for the softmax denominator).
  C: per head pair (row bases 0/64 of shared tiles so score matmuls of a
     pair run on disjoint PE row groups): banded scores for a 256-query
     superblock live in one [128, 1536] PSUM tile (3 banks, region layout
     [b|a|d|c] per head chosen so no matmul output crosses a bank).
     The causal/band masks are applied as additive -30000 matmuls against
     precomputed triangular lhsT tiles with identity rhs (no gpsimd/DVE in
     the chain), then ONE exp activation per (sb, pair) covers both heads.
     att@v accumulates [v|1] so row 64 of yts is the softmax denominator.
  D: reciprocal of the denominators via a [32,128] bounce tile (DVE recip
     is 8 cyc/elem; never run it on a [1, T] row), broadcast via K=1
     matmuls pair-packed into [128, 512] PSUM, normalize on DVE.
  E: out = y_norm @ W_p, K=128 per pair, accumulated over the 2 pairs;
     fp16 partials stream to DRAM per 128-row block.
"""

import numpy as np

B, T, C = 2, 2048, 1024
N_HEAD = 16
MEMORY = 256
D = 64           # head dim
G = 4            # head groups (tensor parallel)
HPG = 4          # heads per group
GC = HPG * D     # 256 columns per group
N_CORES = 8
TB = T // 128    # 16 row blocks
SB = T // 256    # 8 query super-blocks
NEG = -30000.0   # additive mask; exp(0.125 * -30000) == 0 in f32

# aux tensor column layout
AUX_ONES = 0          # [:, 0:68] ones (vplus ones col + bc ones row)
AUX_I = 68            # [:, 68:196] identity, [:, 196:324] identity (I2)
AUX_UP = 324          # [:, 324:452] lhsT for "keep p>=f" mask (strict lower tri = NEG)
AUX_LO = 452          # [:, 452:580] lhsT for "keep p<=f" mask (strict upper tri = NEG)
AUX_W = 580

_PROGRAM_CACHE = {}


def _emit(tc, nc, xT, wqk, wv, wp, aux, out, debug=None):
    import concourse.mybir as mybir

    f32 = mybir.dt.float32
    f16 = mybir.dt.float16

    from contextlib import ExitStack

    ctx = ExitStack()
    with ctx:
        const = ctx.enter_context(tc.tile_pool(name="const", bufs=1))
        wpool = ctx.enter_context(tc.tile_pool(name="wpool", bufs=1))
        xpool = ctx.enter_context(tc.tile_pool(name="xpool", bufs=1))
        qkt_pool = ctx.enter_context(tc.tile_pool(name="qkt", bufs=1))
        vplus_pool = ctx.enter_context(tc.tile_pool(name="vplus", bufs=1))
        expst_pool = ctx.enter_context(tc.tile_pool(name="expst", bufs=3))
        ytpool = ctx.enter_context(tc.tile_pool(name="yt", bufs=1))
        outsb_pool = ctx.enter_context(tc.tile_pool(name="outsb", bufs=3))
        # PSUM: "st" 2 x 3 banks + "bank" 2 x 1 bank = 8 banks
        ps_st = ctx.enter_context(tc.tile_pool(name="ps_st", bufs=2, space="PSUM"))
        ps_bk = ctx.enter_context(tc.tile_pool(name="ps_bk", bufs=2, space="PSUM"))

        # ---- aux constants (ones / identity / mask triangles) ----
        aux_sb = const.tile([128, AUX_W], f16, name="aux", tag="aux")
        nc.gpsimd.dma_start(aux_sb[:], aux[:])
        ones_row = aux_sb[0:1, 0:64]
        ident = aux_sb[:, AUX_I:AUX_I + 128]
        ident2 = aux_sb[:, AUX_I:AUX_I + 256]
        mask_up = aux_sb[:, AUX_UP:AUX_UP + 128]
        mask_lo = aux_sb[:, AUX_LO:AUX_LO + 128]

        # ---- HAM pre-warm: dummy matmuls on zeros while input DMA runs ----
        # The PE clock sits at 1.2 GHz until ~3.4us of sustained activity;
        # burn the DMA prefix warming it so phase A runs at 2.4 GHz.
        wz = const.tile([128, 512], f16, name="warmz", tag="warmz")
        nc.vector.memzero(wz[:])

        # ---- input loads: xT + wqk + wv chunk-interleaved so phase A
        # streams and B's weights are resident before B starts ----
        xT_sb, wqk_sb, wv_sb = [], [], []
        wp_sb = []
        for k in range(8):
            qa = nc.sync if k % 2 == 0 else nc.scalar
            qb = nc.scalar if k % 2 == 0 else nc.sync
            t = xpool.tile([128, T], f16, name=f"xT{k}", tag=f"xT{k}")
            qa.dma_start(t[:], xT[k * 128:(k + 1) * 128, :])
            xT_sb.append(t)
            t = wpool.tile([128, 2 * GC], f16, name=f"wqk{k}", tag=f"wqk{k}")
            qb.dma_start(t[:], wqk[k * 128:(k + 1) * 128, :])
            wqk_sb.append(t)
            t = wpool.tile([128, GC], f16, name=f"wv{k}", tag=f"wv{k}")
            qb.dma_start(t[:], wv[k * 128:(k + 1) * 128, :])
            wv_sb.append(t)
            if k < 2:
                t = wpool.tile([128, C], f16, name=f"wp{k}", tag=f"wp{k}")
                nc.gpsimd.dma_start(t[:], wp[k * 128:(k + 1) * 128, :])
                wp_sb.append(t)

        # ---- phase A: qkT[512, T], k-outer with 8 concurrent chains ----
        qkT_sb = []
        for m in range(4):
            t = qkt_pool.tile([128, T], f16, name=f"qkT{m}", tag=f"qkT{m}")
            qkT_sb.append(t)

        def emit_A(ms, warm=False):
            # 8 chains: (m in ms) x (t4 in 0..3); chain (m, t4) lives in a
            # fixed 512-col region of an st slot (3 chains) or a bank slot.
            st0 = ps_st.tile([128, 1536], f32, name="psA0", tag="st")
            st1 = ps_st.tile([128, 1536], f32, name="psA1", tag="st")
            bk0 = ps_bk.tile([128, 512], f32, name="psA2", tag="bank")
            bk1 = ps_bk.tile([128, 512], f32, name="psA3", tag="bank")
            if warm:
                # HAM pre-warm: dummy matmuls on zeros while the input DMA
                # streams in (PE clock sits at 1.2 GHz until ~3.4us of
                # sustained activity). Output lands in an A-chain region
                # that the real chain re-initializes via start=True.
                for i in range(14):
                    nc.tensor.matmul(st0[:, 0:512], wz[:, 0:128], wz[:],
                                     start=True, stop=True,
                                     skip_group_check=True)
            regions = {}
            for mi, m in enumerate(ms):
                stt, bkt = (st0, bk0) if mi == 0 else (st1, bk1)
                for t4 in range(4):
                    if t4 < 3:
                        regions[(m, t4)] = stt[:, t4 * 512:(t4 + 1) * 512]
                    else:
                        regions[(m, t4)] = bkt[:]
            for k in range(8):
                for m in ms:
                    for t4 in range(4):
                        nc.tensor.matmul(
                            regions[(m, t4)],
                            wqk_sb[k][:, m * 128:(m + 1) * 128],
                            xT_sb[k][:, t4 * 512:(t4 + 1) * 512],
                            start=(k == 0),
                            stop=(k == 7),
                        )
            for i, ((m, t4), reg) in enumerate(regions.items()):
                eng = nc.scalar.copy if i % 2 == 0 else nc.vector.tensor_copy
                eng(qkT_sb[m][:, t4 * 512:(t4 + 1) * 512], reg)

        emit_A((0, 2), warm=True)
        emit_A((1, 3))

        # ---- phase B: v[T, 256] (+ ones col) ----
        vplus_sb = []
        for tb in range(TB):
            ps = ps_bk.tile([128, GC], f32, name="psB", tag="bank")
            for k in range(8):
                nc.tensor.matmul(
                    ps[:],
                    xT_sb[k][:, tb * 128:(tb + 1) * 128],
                    wv_sb[k][:],
                    start=(k == 0),
                    stop=(k == 7),
                )
            vp = vplus_pool.tile([128, HPG, D + 1], f16, name=f"vplus{tb}",
                                 tag=f"vplus{tb}")
            eng = nc.scalar.copy if tb % 2 == 0 else nc.vector.tensor_copy
            eng(vp[:, :, 0:D], ps[:].rearrange("p (h d) -> p h d", h=HPG))
            q = nc.sync if tb % 2 == 0 else nc.scalar
            q.dma_start(
                vp[:, :, D:D + 1],
                aux[:, 0:HPG].rearrange("p (h o) -> p h o", o=1),
            )
            vplus_sb.append(vp)

        # per-head views into qkT: q rows = h*64.., k rows = 256 + h*64..
        def qT_h(h):
            return qkT_sb[h // 2][(h % 2) * 64:(h % 2) * 64 + 64, :]

        def kT_h(h):
            return qkT_sb[2 + h // 2][(h % 2) * 64:(h % 2) * 64 + 64, :]

        # region layout per head within the [128, 1536] score tile:
        #   b: +0:256 (tkb=2sb-1, all 256 queries; right half masked UP)
        #   a: +256:384 (tkb=2sb-2, queries 0:128; masked UP)
        #   d: +384:512 (tkb=2sb+1, queries 128:256; masked LO)
        #   c: +512:768 (tkb=2sb, all queries; left half masked LO)
        yt_sb = [None] * HPG
        ytn_sb = []
        for pr in range(2):
            t = ytpool.tile([128, T], f16, name=f"ytn{pr}", tag=f"ytn{pr}")
            ytn_sb.append(t)
        rt_sb = [
            const.tile([16, 128], f32, name=f"rt{p}", tag=f"rt{p}")
            for p in range(2)
        ]
        rtf_sb = [
            const.tile([16, 128], f16, name=f"rtf{p}", tag=f"rtf{p}")
            for p in range(2)
        ]
        rrow_sb = [
            const.tile([1, T], f16, name=f"rrow{h}", tag=f"rrow{h}")
            for h in range(HPG)
        ]

        for h in range(HPG):
            yt_sb[h] = ytpool.tile([65, T], f32, name=f"yt{h}", tag=f"yt{h}")

        def emit_C(pr, half, fill=None):
            heads = (2 * pr, 2 * pr + 1)
            for sb in range(half * 4, half * 4 + 4):
                if fill is not None:
                    fill(sb - half * 4)
                q0 = sb * 256
                st = ps_st.tile([128, 1536], f32, name="st", tag="st")
                # score matmuls, head pair interleaved (disjoint PE rows).
                # start=True clears has_written for the WHOLE PSUM bank, so
                # it must be set exactly on the first MM touching each bank.
                started = set()

                def st_mm(c0, nn, lhsT, rhs, stop=False):
                    bank = c0 // 512
                    nc.tensor.matmul(
                        st[:, c0:c0 + nn], lhsT, rhs,
                        start=(bank not in started), stop=stop,
                        skip_group_check=True,
                    )
                    started.add(bank)

                steps = []          # (col_off, N, tkb, q_off, q_n)
                if sb > 0:
                    steps.append((0, 256, 2 * sb - 1, 0, 256))
                    steps.append((256, 128, 2 * sb - 2, 0, 128))
                steps.append((384, 128, 2 * sb + 1, 128, 128))
                steps.append((512, 256, 2 * sb, 0, 256))
                for co, nn, tkb, qo, qn in steps:
                    for hi, h in enumerate(heads):
                        st_mm(
                            hi * 768 + co, nn,
                            kT_h(h)[:, tkb * 128:(tkb + 1) * 128],
                            qT_h(h)[:, q0 + qo:q0 + qo + qn],
                        )
                # additive -30000 band/causal masks via identity matmuls;
                # grouped by lhsT so the PE loads each triangle once
                if sb > 0:
                    st_mm(128, 256, mask_up, ident2)
                    st_mm(896, 128, mask_up, ident)
                    st_mm(1024, 128, mask_up, ident)
                st_mm(384, 128, mask_lo, ident)
                st_mm(512, 128, mask_lo, ident)
                st_mm(1152, 256, mask_lo, ident2, stop=True)
                # exp over the whole pair in one activation
                expst = expst_pool.tile([128, 1536], f16, name="expst",
                                        tag="expst")
                if sb > 0:
                    nc.scalar.activation(
                        expst[:], st[:],
                        mybir.ActivationFunctionType.Exp, scale=0.125,
                    )
                else:
                    for hi in range(2):
                        nc.scalar.activation(
                            expst[:, hi * 768 + 384:hi * 768 + 768],
                            st[:, hi * 768 + 384:hi * 768 + 768],
                            mybir.ActivationFunctionType.Exp, scale=0.125,
                        )
                # att @ [v|1]
                for hi, h in enumerate(heads):
                    yts = ps_bk.tile([65, 256], f32, name="yts", tag="bank")
                    av = []         # (col_off, N, tkb, out_off)
                    if sb > 0:
                        av.append((0, 256, 2 * sb - 1, 0))
                    av.append((512, 256, 2 * sb, 0))
                    if sb > 0:
                        av.append((256, 128, 2 * sb - 2, 0))
                    av.append((384, 128, 2 * sb + 1, 128))
                    for j, (co, nn, tkb, oo) in enumerate(av):
                        nc.tensor.matmul(
                            yts[:, oo:oo + nn],
                            vplus_sb[tkb][:, h, :],
                            expst[:, hi * 768 + co:hi * 768 + co + nn],
                            start=(j == 0),
                            stop=(j == len(av) - 1),
                        )
                    nc.vector.tensor_copy(
                        yt_sb[h][:, q0:q0 + 256], yts[:]
                    )

        def emit_D_recip(pr, half):
            heads = (2 * pr, 2 * pr + 1)
            hT = T // 2
            rt = rt_sb[pr]
            for h in heads:
                r0 = (h % 2) * 8
                q = nc.sync if h % 2 == 0 else nc.scalar
                q.dma_start(rt[r0:r0 + 8, :],
                            yt_sb[h][64:65, half * hT:(half + 1) * hT])
            with nc.allow_low_precision(reason="softmax denom reciprocal"):
                nc.vector.reciprocal(rtf_sb[pr][0:16, :], rt[0:16, :])
            for h in heads:
                r0 = (h % 2) * 8
                q = nc.sync if h % 2 == 0 else nc.scalar
                q.dma_start(rrow_sb[h][0:1, half * hT:(half + 1) * hT],
                            rtf_sb[pr][r0:r0 + 8, :])

        def emit_D_norm(pr, t4):
            heads = (2 * pr, 2 * pr + 1)
            bc = ps_bk.tile([128, 512], f32, name="bc", tag="bank")
            for h in heads:
                p0 = (h % 2) * 64
                nc.tensor.matmul(
                    bc[p0:p0 + 64, :],
                    ones_row,
                    rrow_sb[h][0:1, t4 * 512:(t4 + 1) * 512],
                    start=True,
                    stop=(h == heads[1]),
                    skip_group_check=True,
                )
            for h in heads:
                p0 = (h % 2) * 64
                nc.vector.tensor_mul(
                    ytn_sb[pr][p0:p0 + 64, t4 * 512:(t4 + 1) * 512],
                    yt_sb[h][0:64, t4 * 512:(t4 + 1) * 512],
                    bc[p0:p0 + 64, :],
                )

        def emit_E_group(units):
            # up to 3 projection chains per (now free) 3-bank st tile so
            # E is paced by the PE, not by PSUM-slot + eviction latency
            ps = ps_st.tile([128, 1536], f32, name="psE", tag="st")
            for j, (tb, nh) in enumerate(units):
                reg = ps[:, j * 512:(j + 1) * 512]
                for pr in range(2):
                    nc.tensor.matmul(
                        reg,
                        ytn_sb[pr][:, tb * 128:(tb + 1) * 128],
                        wp_sb[pr][:, nh * 512:(nh + 1) * 512],
                        start=(pr == 0),
                        stop=(pr == 1),
                        skip_group_check=True,
                    )
            for j, (tb, nh) in enumerate(units):
                ob = outsb_pool.tile([128, 512], f16, name="outsb", tag="outsb")
                if (tb + nh) % 2 == 0:
                    nc.scalar.copy(ob[:], ps[:, j * 512:(j + 1) * 512])
                else:
                    nc.vector.tensor_copy(ob[:], ps[:, j * 512:(j + 1) * 512])
                qo = nc.sync if (tb * 2 + nh) % 2 == 0 else nc.scalar
                qo.dma_start(
                    out[tb * 128:(tb + 1) * 128, nh * 512:(nh + 1) * 512], ob[:]
                )

        def emit_E_range(units):
            for g in range(0, len(units), 3):
                emit_E_group(units[g:g + 3])

        # Half-pipelined: each half's reciprocal + normalize is emitted as a
        # light fill inside the NEXT C phase (one bc matmul + 2 DVE muls per
        # filled superblock), so by the time phase E starts everything except
        # the last pair-half's normalize is already done.
        def norm_fill(jobs):
            return lambda i: [emit_D_norm(pr, t4) for pr, t4 in jobs.get(i, [])]

        emit_C(0, 0)
        emit_D_recip(0, 0)
        emit_C(1, 0)
        emit_D_recip(1, 0)
        emit_C(0, 1, fill=norm_fill({1: [(0, 0)], 2: [(0, 1)], 3: [(1, 0)]}))
        emit_D_recip(0, 1)
        emit_C(1, 1, fill=norm_fill({1: [(1, 1)], 2: [(0, 2)], 3: [(0, 3)]}))
        emit_D_recip(1, 1)
        units = [(tb, nh) for tb in range(TB) for nh in range(2)]
        emit_E_range(units[0:6])
        emit_D_norm(1, 2)
        emit_E_range(units[6:12])
        emit_D_norm(1, 3)
        emit_E_range(units[12:32])

        if debug is not None:
            d_qkT, d_vplus, d_yt, d_rrow, d_ytn = debug
            for m in range(4):
                nc.gpsimd.dma_start(d_qkT[m * 128:(m + 1) * 128, :],
                                    qkT_sb[m][:])
            for tb in range(TB):
                nc.gpsimd.dma_start(
                    d_vplus[tb * 128:(tb + 1) * 128, :],
                    vplus_sb[tb][:].rearrange("p h d -> p (h d)"),
                )
            for h in range(HPG):
                nc.gpsimd.dma_start(d_yt[h * 65:(h + 1) * 65, :], yt_sb[h][:])
                nc.gpsimd.dma_start(d_rrow[h:h + 1, :], rrow_sb[h][:])
            for pr in range(2):
                nc.gpsimd.dma_start(d_ytn[pr * 128:(pr + 1) * 128, :],
                                    ytn_sb[pr][:])


def build_program():
    key = "v12"
    if key in _PROGRAM_CACHE:
        return _PROGRAM_CACHE[key]
    import concourse.bacc as bacc
    import concourse.mybir as mybir
    import concourse.tile as tile

    f16 = mybir.dt.float16
    nc = bacc.Bacc("TRN2", target_bir_lowering=False, debug=False,
                   num_devices=N_CORES)
    xT = nc.dram_tensor("xT", [C, T], f16, kind="ExternalInput").ap()
    wqk = nc.dram_tensor("wqk", [C, 2 * GC], f16, kind="ExternalInput").ap()
    wv = nc.dram_tensor("wv", [C, GC], f16, kind="ExternalInput").ap()
    wp = nc.dram_tensor("wp", [GC, C], f16, kind="ExternalInput").ap()
    aux = nc.dram_tensor("aux", [128, AUX_W], f16, kind="ExternalInput").ap()
    out = nc.dram_tensor("out", [T, C], f16, kind="ExternalOutput").ap()
    with tile.TileContext(nc) as tc:
        _emit(tc, nc, xT, wqk, wv, wp, aux, out)
    nc.compile()
    _PROGRAM_CACHE[key] = nc
    return nc


def _make_aux():
    aux = np.zeros((128, AUX_W), dtype=np.float32)
    aux[:, 0:68] = 1.0
    i = np.arange(128)
    aux[i, AUX_I + i] = 1.0
    aux[i, AUX_I + 128 + i] = 1.0
    r = i[:, None]
    c = i[None, :]
    aux[:, AUX_UP:AUX_UP + 128] = np.where(c < r, NEG, 0.0)
    aux[:, AUX_LO:AUX_LO + 128] = np.where(c > r, NEG, 0.0)
    return aux.astype(np.float16)


def make_in_maps(x, W_attn, W_proj):
    x = np.asarray(x, dtype=np.float32)
    W_attn = np.asarray(W_attn, dtype=np.float32)
    W_proj = np.asarray(W_proj, dtype=np.float32)
    cast = lambda a: np.ascontiguousarray(a, dtype=np.float16)
    xTs = [cast(x[b].T) for b in range(B)]
    aux = _make_aux()
    in_maps = []
    for c in range(N_CORES):
        b, g = divmod(c, G)
        q_cols = W_attn[:, g * GC:(g + 1) * GC]
        k_cols = W_attn[:, C + g * GC:C + (g + 1) * GC]
        v_cols = W_attn[:, 2 * C + g * GC:2 * C + (g + 1) * GC]
        in_maps.append({
            "xT": xTs[b],
            "wqk": cast(np.concatenate([q_cols, k_cols], axis=1)),
            "wv": cast(v_cols),
            "wp": cast(W_proj[g * GC:(g + 1) * GC, :]),
            "aux": aux,
        })
    return in_maps


def gather(results):
    out = np.zeros((B, T, C), dtype=np.float32)
    for c, res in enumerate(results):
        b = c // G
        out[b] += np.asarray(res["out"], dtype=np.float32)
    return out


def kernel(x, W_attn, W_proj, dtype=None, trace=False):
    from concourse import bass_utils

    nc = build_program()
    in_maps = make_in_maps(x, W_attn, W_proj)
    r = bass_utils.run_bass_kernel_spmd(
        nc, in_maps, core_ids=list(range(N_CORES)), trace=trace
    )
    out = gather(r.results)
    if trace:
        kernel.last_results = r
    return out
